# revision 1
# baseline (speedup 1.0000x reference)
"""AdaptiveSampler: direct DRAM->DRAM per-frame DMAs with register-patched
source offsets.

Per core, 64 output frames (some invalid -> skipped). Program slot k writes
out rows [4k, 4k+4) from xz at a dynamic element offset loaded from SBUF.
Three engines issue the DMAs (sync/SP, scalar/Activation, gpsimd/Pool) so
per-DMA issue cost (~0.7-0.9us) overlaps the transfers. Invalid frames use an
out-of-bounds offset with bounds_check="skip_entire_dma" (semaphore still
increments). Host assigns (batch, frame) pairs to program slots and
inverse-permutes on readback.
"""

import os

import numpy as np

import concourse.bass as bass
import concourse.mybir as mybir
from concourse.bass_utils import run_bass_kernel_spmd

B, T, C, H, W = 32, 64, 3, 112, 112
AOT = 4
NCORES = 8
BL = B // NCORES
CHW = C * H * W              # 37632
SUB = 4
SUBLEN = CHW // SUB          # 9408
NROWS_IN = BL * T * SUB      # 1024
FRAMES_OUT = BL * AOT * AOT  # 64
NROWS_OUT = FRAMES_OUT * SUB # 256
OOB_EL = NROWS_IN * SUBLEN   # element offset just past the end -> whole-DMA skip

# frame slots per issuing engine
N_SP = 22
N_ACT = 21
N_POOL = FRAMES_OUT - N_SP - N_ACT

TRACE = False
RUN_KWARGS = {}
LAST_RESULT = None

_graph_cache = {}


def _build_graph():
    nc = bass.Bass()
    xz = nc.declare_dram_parameter("xz", [NROWS_IN, SUBLEN], mybir.dt.float32, isOutput=False)
    idx = nc.declare_dram_parameter("idx", [1, FRAMES_OUT], mybir.dt.int32, isOutput=False)
    out = nc.declare_dram_parameter("out", [NROWS_OUT, SUBLEN], mybir.dt.float32, isOutput=True)

    FRAME_AP = [[SUBLEN, SUB], [1, SUBLEN]]

    with (
        nc.sbuf_tensor("idxs", [1, FRAMES_OUT], mybir.dt.int32) as idxs,
        nc.semaphore("s_idx") as s_idx,
        nc.semaphore("s_sp") as s_sp,
        nc.semaphore("s_act") as s_act,
        nc.semaphore("s_pool") as s_pool,
        nc.Block() as block,
    ):
        def issue(eng, reg, frames, sem):
            first = True
            for k in frames:
                ld = eng.reg_load(reg, idxs[0:1, k:k + 1])
                if first:
                    ld._wait_ge(s_idx, 16)
                    first = False
                val = eng.snap(reg)
                src = bass.AP(xz, val, [list(d) for d in FRAME_AP])
                eng.dma_start(
                    out[SUB * k:SUB * (k + 1), :],
                    src,
                    bounds_check="skip_entire_dma",
                ).then_inc(sem, 16)
            eng.wait_ge(sem, 16 * len(frames))

        @block.sync
        def _(sync):
            sync.dma_start(out=idxs[:, :], in_=idx[:, :]).then_inc(s_idx, 16)
            with sync.register() as reg:
                issue(sync, reg, range(N_SP), s_sp)

        @block.scalar
        def _(act):
            with act.register() as reg:
                issue(act, reg, range(N_SP, N_SP + N_ACT), s_act)

        @block.gpsimd
        def _(gpsimd):
            with gpsimd.register() as reg:
                issue(gpsimd, reg, range(N_SP + N_ACT, FRAMES_OUT), s_pool)

    return nc


def _get_graph():
    if "nc" not in _graph_cache:
        _graph_cache["nc"] = _build_graph()
    return _graph_cache["nc"]


def _frame_indices(dt, delta_t):
    import jax
    import jax.numpy as jnp

    with jax.default_device(jax.devices("cpu")[0]):
        dtj = jnp.asarray(np.asarray(dt, dtype=np.float32))
        dlj = jnp.asarray(np.asarray(delta_t, dtype=np.float32))
        anchor_t = (T - 1) / 2.0
        dts = dtj * anchor_t + anchor_t
        deltas = (T / (AOT - 1) - 1.0) * dlj + 1.0
        grid = jnp.arange(AOT, dtype=jnp.float32)
        mu = dts[:, :, None] + (grid[None, None, :] - (AOT - 1) / 2.0) * deltas[:, :, None]
        idxf = np.asarray(jnp.ceil(mu))
    valid = (idxf >= 0) & (idxf <= T - 1)
    t_idx = np.where(valid, idxf, 0).astype(np.int64)
    return t_idx.reshape(B, AOT * AOT), valid.reshape(B, AOT * AOT)


def _plan(t_flat, v_flat):
    """Per-core slot assignment. Returns packs, per-core idx arrays and
    slot_of_frame permutations."""
    vcnt = v_flat.sum(axis=1)
    loads = [0] * NCORES
    packs = [[] for _ in range(NCORES)]
    for b in sorted(range(B), key=lambda b: -vcnt[b]):
        m = min((m for m in range(NCORES) if len(packs[m]) < BL), key=lambda m: loads[m])
        packs[m].append(b)
        loads[m] += vcnt[b]

    plans = []
    for m in range(NCORES):
        batches = np.asarray(packs[m])
        # logical frames: (bl, j*4+g) -> src subrow (bl*T + t)*SUB, valid flag
        src_el = np.full(FRAMES_OUT, OOB_EL, np.int64)
        valid = np.zeros(FRAMES_OUT, bool)
        for bl in range(BL):
            b = batches[bl]
            for f in range(AOT * AOT):
                lf = bl * AOT * AOT + f
                if v_flat[b, f]:
                    src_el[lf] = (bl * T + t_flat[b, f]) * SUB * SUBLEN
                    valid[lf] = True
        # assign logical frames to program slots: spread valid frames evenly
        # over the three issue engines (and thus roughly over time)
        order = np.argsort(~valid, kind="stable")  # valid first
        slots_by_engine = [list(range(N_SP)), list(range(N_SP, N_SP + N_ACT)),
                           list(range(N_SP + N_ACT, FRAMES_OUT))]
        slot_seq = []
        i = 0
        while any(slots_by_engine):
            e = i % 3
            if slots_by_engine[e]:
                slot_seq.append(slots_by_engine[e].pop(0))
            i += 1
        slot_of_frame = np.empty(FRAMES_OUT, np.int64)
        for lf, slot in zip(order, slot_seq):
            slot_of_frame[lf] = slot
        idx_np = np.full((1, FRAMES_OUT), OOB_EL, np.int32)
        idx_np[0, slot_of_frame] = src_el
        plans.append((batches, idx_np, slot_of_frame))
    return plans


def kernel(x, dt, delta_t):
    global LAST_RESULT
    x = np.ascontiguousarray(np.asarray(x), dtype=np.float32)
    t_flat, v_flat = _frame_indices(dt, delta_t)
    plans = _plan(t_flat, v_flat)

    in_maps = []
    for batches, idx_np, _ in plans:
        xs = np.ascontiguousarray(x[batches]).reshape(NROWS_IN, SUBLEN)
        in_maps.append({"xz": xs, "idx": idx_np})

    if TRACE:
        os.environ.pop("BASS_NEVER_TRACE", None)
    else:
        os.environ["BASS_NEVER_TRACE"] = "1"

    nc = _get_graph()
    last_err = None
    for attempt in range(3):
        try:
            LAST_RESULT = run_bass_kernel_spmd(
                nc, in_maps, core_ids=list(range(NCORES)), trace=TRACE, **RUN_KWARGS
            )
            break
        except Exception as e:
            last_err = e
            import time
            time.sleep(5 * (attempt + 1))
    else:
        raise last_err

    out_full = np.zeros((B, AOT * AOT, C, H, W), np.float32)
    for m, r in enumerate(LAST_RESULT.results):
        batches, _, slot_of_frame = plans[m]
        ro = r["out"].reshape(FRAMES_OUT, CHW)
        out_full[np.asarray(batches)] = ro[slot_of_frame].reshape(BL, AOT * AOT, C, H, W)
    return out_full

